# revision 4
# baseline (speedup 1.0000x reference)
"""CoreAttention Trainium2 Bass kernel.

Full inputs -> full output; internally shards (batch, head-group) across 8
NeuronCores: core c handles batch c//4, heads 4*(c%4) .. 4*(c%4)+4.

Per-core algorithm (per head, seq=2048, d=128):
  - scores are computed TRANSPOSED: S^T[k, q] = (K^T).T @ (Q^T) on the PE,
    so that softmax probabilities come out directly in the [k, q] layout that
    the second matmul (context = P @ V) needs as its stationary operand --
    no per-tile transpose of the 2048x2048 probability matrix.
  - softmax skips max-subtraction (logits ~ N(0,1); exp is safe in fp32) and
    the row sums come for free from a ones-column appended to V.  Masked
    entries are zeroed exactly after exp (matching the reference where
    exp(-10000 - max) underflows to 0), and normalization happens on the
    [q, 128] context output via a per-row reciprocal.
  - the boolean mask is loaded transposed via the DMA XBAR transpose, which
    only supports 2-byte elements: the [q, k] uint8 mask is viewed as uint16
    pairs along k, transposed to [k/2, q], and unpacked with bitwise AND into
    per-k-tile predicates.  This interleaves even/odd keys; K and V rows are
    loaded with the same even/odd permutation so everything stays consistent
    (softmax over k is permutation invariant).
  - PE operands are fp16 (1 cycle/row); accumulation is fp32 in PSUM.
"""

from contextlib import ExitStack

import numpy as np

import concourse.bacc as bacc
from concourse import mybir
import concourse.tile as tile
from concourse.bass_utils import run_bass_kernel_spmd
from concourse.masks import make_identity

S, B, H, D = 2048, 2, 16, 128
HPC = 4  # heads per core
N_CORES = 8
P = 128
NT = S // P  # 16 key/query tiles
SCALE = float(1.0 / np.sqrt(D))  # norm_factor = sqrt(d) * layer_number(=1)

f32 = mybir.dt.float32
f16 = mybir.dt.float16
u16 = mybir.dt.uint16

Exp = mybir.ActivationFunctionType.Exp
AND = mybir.AluOpType.bitwise_and


def _emit(ctx, tc, q_d, k_d, v_d, m_d, o_d):
    nc = tc.nc
    const = ctx.enter_context(tc.tile_pool(name="const", bufs=1))
    maskt = ctx.enter_context(tc.tile_pool(name="maskt", bufs=2))
    predp = ctx.enter_context(tc.tile_pool(name="pred", bufs=1))
    ktp = ctx.enter_context(tc.tile_pool(name="kt", bufs=2))
    qtp = ctx.enter_context(tc.tile_pool(name="qt", bufs=2))
    vpp = ctx.enter_context(tc.tile_pool(name="vp", bufs=2))
    stg = ctx.enter_context(tc.tile_pool(name="stg", bufs=1))
    ptp = ctx.enter_context(tc.tile_pool(name="pt", bufs=2))
    outp = ctx.enter_context(tc.tile_pool(name="outq", bufs=2))
    rcp = ctx.enter_context(tc.tile_pool(name="rc", bufs=2))
    ps_s = ctx.enter_context(tc.tile_pool(name="ps_s", bufs=2, space="PSUM"))
    ps_m = ctx.enter_context(tc.tile_pool(name="ps_m", bufs=2, space="PSUM"))

    ident = const.tile([P, P], f32)
    make_identity(nc, ident[:])
    zeros = const.tile([P, S // 2], f16)
    nc.gpsimd.memset(zeros[:], 0.0)

    # ---- mask: XBAR-transpose uint16 chunks, unpack into per-k-tile
    # predicates.  pred[:, t, q] is nonzero iff key (tile t, row p) is masked
    # for query q, with key(t=2a+m, p) = 256a + 2p + m.
    pred = predp.tile([P, NT, S], u16)
    for c in range(NT // 2):
        mt = maskt.tile([P, S], u16)
        nc.sync.dma_start_transpose(mt[:], m_d[:, c * P:(c + 1) * P])
        nc.vector.tensor_scalar(pred[:, 2 * c, :], mt[:], 0x0001, None, AND)
        nc.vector.tensor_scalar(pred[:, 2 * c + 1, :], mt[:], 0x0100, None, AND)

    # DRAM views.  K/V use the same even/odd key permutation as the mask:
    # partition-first so DMA flattening order matches the SBUF tile layout.
    k_r = k_d.rearrange("(a p m) h d -> p a m h d", p=P, m=2)
    v_r = v_d.rearrange("(a p m) h d -> p a m h d", p=P, m=2)
    q_r = q_d.rearrange("(j p) h d -> p j h d", p=P)
    o_r = o_d.rearrange("(qd jj p) h d -> qd p jj h d", jj=4, p=P)

    staged = {}
    head_res = {}

    def load(i):
        qs = stg.tile([P, NT, D], f32, tag="qs")
        ks = stg.tile([P, NT // 2, 2, D], f32, tag="ks")
        vs = stg.tile([P, NT // 2, 2, D], f32, tag="vs")
        nc.sync.dma_start(qs[:], q_r[:, :, i, :])
        for m in range(2):
            nc.sync.dma_start(ks[:, :, m, :], k_r[:, :, m, i, :])
            nc.sync.dma_start(vs[:, :, m, :], v_r[:, :, m, i, :])
        staged[i] = (qs, ks, vs)

    def prep_chunks(i):
        """Emit-chunk closures: 4 K-transpose quads, 4 Q-transpose quads,
        V convert + ones column."""
        qs, ks, vs = staged[i]
        KT = ktp.tile([P, NT, P], f16)
        QT = qtp.tile([P, S], f16)
        VP = vpp.tile([P, NT // 2, 2, D + 1], f16)
        head_res[i] = (KT, QT, VP)
        chunks = []

        def k_quad(u):
            psq = ps_m.tile([P, 512], f32, tag="tp")
            for w in range(4):
                t = 4 * u + w
                nc.tensor.transpose(
                    psq[:, P * w:P * (w + 1)], ks[:, t // 2, t % 2, :], ident[:])
            nc.vector.tensor_copy(KT[:, 4 * u:4 * u + 4, :], psq[:])

        def q_quad(u):
            psq = ps_m.tile([P, 512], f32, tag="tp")
            for w in range(4):
                nc.tensor.transpose(
                    psq[:, P * w:P * (w + 1)], qs[:, 4 * u + w, :], ident[:])
            nc.vector.tensor_copy(QT[:, 512 * u:512 * (u + 1)], psq[:])

        def v_conv():
            nc.gpsimd.tensor_copy(VP[:, :, :, 0:D], vs[:])
            nc.gpsimd.memset(VP[:, :, :, D:D + 1], 1.0)

        for u in range(4):
            chunks.append(lambda u=u: k_quad(u))
        for u in range(4):
            chunks.append(lambda u=u: q_quad(u))
        chunks.append(v_conv)
        return chunks

    def mm1_step(i, hh, t, PT):
        KT, QT, VP = head_res[i]
        q0 = (S // 2) * hh
        ps = ps_s.tile([P, 1024], f32)
        nc.tensor.matmul(ps[:, 0:512], KT[:, t, :], QT[:, q0:q0 + 512],
                         start=True, stop=True)
        nc.tensor.matmul(ps[:, 512:1024], KT[:, t, :], QT[:, q0 + 512:q0 + 1024],
                         start=True, stop=True)
        nc.scalar.activation(PT[:, t, :], ps[:], Exp, scale=SCALE)
        nc.vector.copy_predicated(PT[:, t, :], pred[:, t, q0:q0 + 1024], zeros[:])

    oq_state = {}

    def mm2_step(prev, jj):
        i, hh, PT = prev
        KT, QT, VP = head_res[i]
        j = 8 * hh + jj  # global q-tile index
        po = ps_m.tile([P, D + 1], f32, tag="o")
        for t in range(NT):
            nc.tensor.matmul(po[:], PT[:, t, P * jj:P * (jj + 1)],
                             VP[:, t // 2, t % 2, :],
                             start=(t == 0), stop=(t == NT - 1))
        rc = rcp.tile([P, 1], f32)
        nc.vector.reciprocal(rc[:], po[:, D:D + 1])
        quad, sub = divmod(j, 4)
        if sub == 0:
            oq_state[i] = outp.tile([P, 4, D], f32, name="oq", tag="oq")
        oq = oq_state[i]
        nc.vector.tensor_scalar_mul(oq[:, sub, :], po[:, 0:D], rc[:])
        if sub == 3:
            nc.sync.dma_start(o_r[quad, :, :, i, :], oq[:])

    # ---- software pipeline over 8 half-heads --------------------------
    halves = [(i, hh) for i in range(HPC) for hh in range(2)]
    load(0)
    for ch in prep_chunks(0):
        ch()
    prev = None
    for (i, hh) in halves:
        PT = ptp.tile([P, NT, S // 2], f16)
        if hh == 0 and i + 1 < HPC:
            load(i + 1)
        pending = prep_chunks(i + 1) if (hh == 1 and i + 1 < HPC) else []
        for x in range(NT):
            mm1_step(i, hh, x, PT)
            if prev is not None and x % 2 == 1:
                mm2_step(prev, x // 2)
            if pending and x >= NT - len(pending):
                pending[x - (NT - len(pending))]()
        prev = (i, hh, PT)
    for jj in range(8):
        mm2_step(prev, jj)


def build_nc():
    nc = bacc.Bacc("TRN2", target_bir_lowering=False, debug=False)
    q_d = nc.dram_tensor("q", [S, HPC, D], f32, kind="ExternalInput").ap()
    k_d = nc.dram_tensor("k", [S, HPC, D], f32, kind="ExternalInput").ap()
    v_d = nc.dram_tensor("v", [S, HPC, D], f32, kind="ExternalInput").ap()
    m_d = nc.dram_tensor("mask16", [S, S // 2], u16, kind="ExternalInput").ap()
    o_d = nc.dram_tensor("out", [S, HPC, D], f32, kind="ExternalOutput").ap()
    with tile.TileContext(nc) as tc, ExitStack() as ctx:
        _emit(ctx, tc, q_d, k_d, v_d, m_d, o_d)
    nc.compile()
    return nc


_nc_cache = None


def get_nc():
    global _nc_cache
    if _nc_cache is None:
        _nc_cache = build_nc()
    return _nc_cache


def make_in_maps(query_layer, key_layer, value_layer, attention_mask):
    q = np.asarray(query_layer, dtype=np.float32)
    k = np.asarray(key_layer, dtype=np.float32)
    v = np.asarray(value_layer, dtype=np.float32)
    m = np.asarray(attention_mask)
    mask16 = [
        np.ascontiguousarray(m[b, 0]).view(np.uint8).reshape(S, S)
        .view(np.uint16) for b in range(B)
    ]
    in_maps = []
    for c in range(N_CORES):
        b, g = divmod(c, HPC)
        hs = slice(HPC * g, HPC * g + HPC)
        in_maps.append({
            "q": np.ascontiguousarray(q[:, b, hs, :]),
            "k": np.ascontiguousarray(k[:, b, hs, :]),
            "v": np.ascontiguousarray(v[:, b, hs, :]),
            "mask16": mask16[b],
        })
    return in_maps


def assemble(results):
    out = np.empty((S, B, H, D), np.float32)
    for c in range(N_CORES):
        b, g = divmod(c, HPC)
        out[:, b, HPC * g:HPC * g + HPC, :] = results[c]["out"]
    return out.reshape(S, B, H * D)


def kernel(query_layer, key_layer, value_layer, attention_mask):
    nc = get_nc()
    in_maps = make_in_maps(query_layer, key_layer, value_layer, attention_mask)
    res = run_bass_kernel_spmd(nc, in_maps, core_ids=list(range(N_CORES)))
    return assemble(res.results)
